# revision 12
# baseline (speedup 1.0000x reference)
"""MLA q/kv projection kernel for Trainium2, 8 NeuronCores, SPMD data-parallel
over the token dimension (512 tokens per core).

Per-core pipeline (v2 — transposed-cq restructure):
  kvmm:  kv[512t, 512] (token-major, x stationary) and kropeT[64, 512t]
         (wkv rope cols stationary) streamed over k; runs first as PE warmup
         while DMA ramps.  kv rmsnorm*gamma + rope token-major, stored.
  cqmm:  cqT[1536, 512t] = wq_a.T @ x.T in 3 phases of 4x128 col-blocks
         (weights stationary, tokens moving) -> PSUM holds cq TRANSPOSED,
         which is exactly mm2's stationary layout: no transposes at all.
         cq is NOT normalized here; evictions cast to bf16 cqT tiles and
         Act squares each tile; 1-row ones-matmuls reduce the squares over
         partitions into ssq[tok,1] per m (pipelined one phase late so PE
         never waits on Act).
  s4:    1/sqrt(ssq/L + eps) per token = the rmsnorm scale, applied at mm2
         eviction (rope is per-token linear, so post-scaling is exact).
         gamma_cq is folded into wq_b on the host.
  mm2:   q[512t, 24576] = cq.T @ wb in ORIGINAL head order, 64 n-tiles of
         384 cols (2 heads each): eviction does nope-cols scalar-mul by s4
         and rope with s4-prescaled cos/sin, then one fully contiguous
         store per (n, m) tile (1536B runs).

Host-side prep: shard+transpose token_x; pre-tile wkv/wq_a/wq_b into
per-DMA-contiguous layouts; fold gamma_cq into wq_b; pack cos/sin as
[c1|c1|c2|c2|s1|s1|s2|s2]x32 so 2-head-fused rope slices line up.
"""

import os

import numpy as np

import concourse.bass as bass
import concourse.tile as tile
from concourse import mybir
from concourse.bass_utils import run_bass_kernel_spmd
from concourse.masks import make_identity
from concourse.vector_clock import ScopedClock, VectorClock

F32 = mybir.dt.float32
BF16 = mybir.dt.bfloat16

N_CORES = 8
T = 4096
TC = T // N_CORES           # 512 tokens per core
MT = TC // 128              # 4 token tiles
H = 7168
KH = H // 128               # 56 contraction tiles for mm1
KH2 = KH // 2               # 28 paired-row loads
L = 1536                    # q latent
KL = L // 128               # 12 contraction tiles for mm2
KL2 = KL // 2
KV_RANK = 512
R = 64                      # rope dims
N_HEADS = 128
QK_NOPE = 128
DN = N_HEADS * (QK_NOPE + R)   # 24576
NW = 512                    # mm2 n-tile width (2 2/3 heads)
NT = DN // NW               # 48 n-tiles
OUTW = DN + KV_RANK + R     # 25152
EPS = 1e-6

# mm2 eviction patterns per (tile_index % 3): head-aligned 192-col periods
# sliced by 512-wide tiles.  rope: (base, nblocks); nope: list of
# (base, nblocks, width).  All blocks stride 192.
MM2_PAT = [
    {"rope": (128, 2), "nope": [(0, 3, 128)]},
    {"rope": (0, 3), "nope": [(64, 2, 128), (448, 1, 64)]},
    {"rope": (64, 3), "nope": [(0, 1, 64), (128, 2, 128)]},
]


def _blocks(ap2d, base, nb, w, stride=192):
    """3D view of a 2D AP: [partitions, nb blocks (elem stride `stride`),
    w contiguous elems] starting at free-offset `base`."""
    return bass.AP(tensor=ap2d.tensor, offset=ap2d.offset + base,
                   ap=[list(ap2d.ap[0]), [stride, nb], [1, w]])


def split_multi_waits(nc, limit=1):
    """Walrus in this toolchain accepts at most one sync-wait command per
    TPB instruction. Hoist extra waits onto single-wait NoOps inserted
    immediately before the offending instruction on the same engine."""
    skip = (mybir.InstAllEngineBarrier, mybir.InstEventSemaphore)
    for f in nc.m.functions:
        for bb in f.blocks:
            new_insts = []
            changed = False
            for inst in bb.instructions:
                si = inst.sync_info
                waits = list(si.on_wait) if si is not None and si.on_wait else []
                if len(waits) > limit and not isinstance(inst, skip):
                    for w in waits[:-limit]:
                        nop = mybir.InstNoOp(
                            name=nc.get_next_instruction_name(),
                            sync_info=mybir.SyncInfo(on_wait=[w], on_update=[]),
                            bass_nofuse=True,
                            engine=inst.engine,
                        )
                        new_insts.append(nop)
                    inst.sync_info = mybir.SyncInfo(
                        on_wait=waits[-limit:], on_update=list(si.on_update))
                    changed = True
                new_insts.append(inst)
            if changed:
                bb.instructions = new_insts
    return nc


class PatchedTC(tile.TileContext):
    """Workaround for the same walrus limit at the kernel tail: the SP Drain
    only accepts ONE sync-wait, while Tile attaches one per active processor.
    Chain single-wait drains instead."""

    def _drain_and_barrier(self, tick_clock, wait_clock):
        nc = self.nc
        gc = tick_clock.global_clock
        nprocs = len(gc)
        procs = [p for p in range(nprocs) if gc[p] > 0] or [0]
        for p in procs:
            d = nc.sync.drain()
            vc = VectorClock([0] * nprocs)
            vc.require_at_least(p, gc[p])
            wait_clock.add_sem_waits(d.ins, ScopedClock({None: vc}))
        nc.all_engine_barrier()
        assert self.sems is not None
        popped = nc._tile_sem_poison_stack.pop()
        assert popped is self._sem_poison
        nc.clear_and_free_semaphores(list(self.sems.allocated().values()))
        nc.all_engine_barrier()


def build_nc(split=True):
    reps = int(os.environ.get("MLA_REPS", "1"))
    wb_bufs = int(os.environ.get("MLA_WB_BUFS", "16"))
    wdq_bufs = int(os.environ.get("MLA_WDQ_BUFS", "6"))
    wdkv_bufs = int(os.environ.get("MLA_WDKV_BUFS", "8"))
    qout_bufs = int(os.environ.get("MLA_QOUT_BUFS", "6"))

    Sq = mybir.ActivationFunctionType.Square
    Sqrt = mybir.ActivationFunctionType.Sqrt

    nc = bass.Bass()
    xt = nc.dram_tensor("xt", [H, TC], BF16, kind="ExternalInput")
    wdkv = nc.dram_tensor("wdkv", [KH2 * 128, 2 * 576], BF16,
                          kind="ExternalInput")
    wdq = nc.dram_tensor("wdq", [3 * KH2 * 128, 2 * 512], BF16,
                         kind="ExternalInput")
    wb = nc.dram_tensor("wb", [NT * KL2 * 128, 2 * NW], BF16,
                        kind="ExternalInput")
    cs = nc.dram_tensor("cs", [TC, 384], F32, kind="ExternalInput")
    gkv = nc.dram_tensor("gkv", [KV_RANK], F32, kind="ExternalInput")
    out = nc.dram_tensor("out", [TC, OUTW], F32, kind="ExternalOutput")

    out_ap = out.ap()

    with PatchedTC(nc) as tc:
        with (
            tc.tile_pool(name="consts", bufs=1) as p_const,
            tc.tile_pool(name="cs", bufs=1) as p_cs,
            tc.tile_pool(name="xt", bufs=1) as p_xt,
            tc.tile_pool(name="wdkv", bufs=wdkv_bufs) as p_wdkv,
            tc.tile_pool(name="wdq", bufs=wdq_bufs) as p_wdq,
            tc.tile_pool(name="cqt", bufs=2) as p_cqt,
            tc.tile_pool(name="sq", bufs=8) as p_sq,
            tc.tile_pool(name="kv", bufs=1) as p_kv,
            tc.tile_pool(name="krT", bufs=1) as p_krT,
            tc.tile_pool(name="stats", bufs=2) as p_stats,
            tc.tile_pool(name="tmp", bufs=2) as p_tmp,
            tc.tile_pool(name="wb", bufs=wb_bufs) as p_wb,
            tc.tile_pool(name="qout", bufs=qout_bufs) as p_qout,
            tc.tile_pool(name="psum", bufs=7, space="PSUM") as p_ps,
            tc.tile_pool(name="psum_ss", bufs=1, space="PSUM") as p_ss,
        ):
            # ---- constants ----
            idf = p_const.tile([64, 64], F32, tag="idf", name="idf")
            make_identity(nc, idf)
            ones1 = p_const.tile([128, 1], BF16, tag="ones1", name="ones1")
            nc.vector.memset(ones1, 1.0)
            eps_t = p_const.tile([128, 1], F32, tag="eps", name="eps_t")
            nc.vector.memset(eps_t, EPS)
            gamma_b = p_const.tile([128, KV_RANK], F32, tag="gamma",
                                   name="gamma_b")

            for _rep in range(reps):
                # ======== phase A: kv latent (token-major) + kropeT ========
                kv_ps = [p_ps.tile([128, 512], F32, tag="ps", name="ps")
                         for _ in range(MT)]
                kr_ps = p_ps.tile([128, 512], F32, tag="ps", name="ps")
                xt_tiles = {}
                cs_sb = []
                for k2 in range(KH2):
                    xt_tiles[k2] = p_xt.tile([128, 2, TC], BF16,
                                             tag=f"xt{k2}", name=f"xt{k2}")
                    nc.sync.dma_start(
                        out=xt_tiles[k2],
                        in_=xt.ap()[k2 * 256:(k2 + 1) * 256, :]
                        .rearrange("(b p) t -> p b t", p=128))
                    wkv_t = p_wdkv.tile([128, 2, 576], BF16, tag="wdkv",
                                        name="wkv_t")
                    nc.sync.dma_start(
                        out=wkv_t,
                        in_=wdkv.ap()[k2 * 128:(k2 + 1) * 128, :]
                        .rearrange("p (b c) -> p b c", c=576))
                    if k2 == 0:
                        # small consts after first big tiles are queued
                        g_ap = gkv.ap()
                        nc.sync.dma_start(
                            out=gamma_b,
                            in_=bass.AP(tensor=g_ap.tensor, offset=g_ap.offset,
                                        ap=[[0, 128]] + [list(p)
                                                         for p in g_ap.ap]))
                        for m in range(MT):
                            t = p_cs.tile([128, 384], F32, tag=f"cs{m}",
                                          name=f"cs{m}")
                            nc.sync.dma_start(
                                out=t,
                                in_=cs.ap()[m * 128:(m + 1) * 128, :])
                            cs_sb.append(t)
                    for b in range(2):
                        k = 2 * k2 + b
                        for m in range(MT):
                            nc.tensor.matmul(
                                kv_ps[m],
                                lhsT=xt_tiles[k2][:, b,
                                                  m * 128:(m + 1) * 128],
                                rhs=wkv_t[:, b, 0:512],
                                start=(k == 0), stop=(k == KH - 1))
                        nc.tensor.matmul(
                            kr_ps[0:64, :],
                            lhsT=wkv_t[:, b, 512:576],
                            rhs=xt_tiles[k2][:, b, :],
                            start=(k == 0), stop=(k == KH - 1))

                # kv rmsnorm * gamma (Act+DVE only; PE moves on to phase B)
                kv_sb = []
                for m in range(MT):
                    kv_m = p_kv.tile([128, KV_RANK + R], F32, tag=f"kv{m}",
                                     name=f"kv{m}")
                    st = p_stats.tile([128, 1], F32, tag=f"st{m}",
                                      name=f"st{m}")
                    scr = p_sq.tile([128, 512], BF16, tag="sq", name="scr")
                    nc.scalar.activation(
                        out=scr, in_=kv_ps[m], func=Sq, accum_out=st)
                    nc.scalar.activation(
                        out=st, in_=st, func=Sqrt,
                        bias=eps_t, scale=1.0 / KV_RANK)
                    nc.vector.reciprocal(out=st, in_=st)
                    nc.vector.tensor_scalar_mul(
                        out=kv_m[:, 0:KV_RANK], in0=kv_ps[m], scalar1=st)
                    nc.vector.tensor_mul(
                        out=kv_m[:, 0:KV_RANK], in0=kv_m[:, 0:KV_RANK],
                        in1=gamma_b)
                    kv_sb.append(kv_m)

                # ======== phase B: cqT in 3 phases of 4 col-blocks ========
                cqT = p_cqt.tile([128, KL, TC], BF16, tag="cqt", name="cqT")
                ssq_sb = p_stats.tile([128, MT], F32, tag="ssq",
                                      name="ssq_sb")
                sq_tiles = {}

                def emit_ssq(p):
                    # 1-row ones-matmuls: ssq[tok,m] = sum_part sq[p]^2.
                    # PSUM allows one open accumulation group per bank, so
                    # per-m groups run sequentially and per-phase partials
                    # accumulate into SBUF.
                    ss_ps = p_ss.tile([128, MT], F32, tag="ss", name="ss_ps")
                    for m in range(MT):
                        for ct in range(4):
                            nc.tensor.matmul(
                                ss_ps[:, m:m + 1],
                                lhsT=sq_tiles[p][ct][:,
                                                     m * 128:(m + 1) * 128],
                                rhs=ones1,
                                start=(ct == 0), stop=(ct == 3))
                    if p == 0:
                        nc.vector.tensor_copy(out=ssq_sb, in_=ss_ps)
                    else:
                        nc.vector.tensor_add(out=ssq_sb, in0=ssq_sb,
                                             in1=ss_ps)

                def emit_krope_tail():
                    # kropeT -> token-major (PE transpose), rope, kv store
                    krT = p_krT.tile([64, 512], F32, tag="krT", name="krT")
                    nc.vector.tensor_copy(out=krT, in_=kr_ps[0:64, :])
                    for m in range(MT):
                        tpk = p_ps.tile([128, 512], F32, tag="ps", name="ps")
                        nc.tensor.transpose(
                            tpk[:, 0:64], krT[:, m * 128:(m + 1) * 128], idf)
                        kv_m = kv_sb[m]
                        x1 = tpk[:, 0:32]
                        x2 = tpk[:, 32:64]
                        c1 = cs_sb[m][:, 0:32]
                        c2 = cs_sb[m][:, 96:128]
                        s1 = cs_sb[m][:, 192:224]
                        s2 = cs_sb[m][:, 288:320]
                        ta = p_tmp.tile([128, 64], F32, tag="ta", name="ta")
                        tb = p_tmp.tile([128, 64], F32, tag="tb", name="tb")
                        nc.vector.tensor_mul(out=ta[:, 0:32], in0=x1, in1=c1)
                        nc.vector.tensor_mul(out=tb[:, 0:32], in0=x2, in1=s1)
                        nc.vector.tensor_sub(
                            out=kv_m[:, KV_RANK:KV_RANK + 32],
                            in0=ta[:, 0:32], in1=tb[:, 0:32])
                        ta2 = p_tmp.tile([128, 64], F32, tag="ta", name="ta")
                        tb2 = p_tmp.tile([128, 64], F32, tag="tb", name="tb")
                        nc.vector.tensor_mul(out=ta2[:, 0:32], in0=x2, in1=c2)
                        nc.vector.tensor_mul(out=tb2[:, 0:32], in0=x1, in1=s2)
                        nc.vector.tensor_add(
                            out=kv_m[:, KV_RANK + 32:KV_RANK + 64],
                            in0=ta2[:, 0:32], in1=tb2[:, 0:32])
                        nc.sync.dma_start(
                            out=out_ap[m * 128:(m + 1) * 128, DN:OUTW],
                            in_=kv_m)

                for p in range(3):
                    cb_ps = [p_ps.tile([128, 512], F32, tag="ps", name="ps")
                             for _ in range(4)]
                    for k2 in range(KH2):
                        wdq_t = p_wdq.tile([128, 2, 512], BF16, tag="wdq",
                                           name="wdq_t")
                        nc.sync.dma_start(
                            out=wdq_t,
                            in_=wdq.ap()[(p * KH2 + k2) * 128:
                                         (p * KH2 + k2 + 1) * 128, :]
                            .rearrange("q (b c) -> q b c", c=512))
                        for b in range(2):
                            k = 2 * k2 + b
                            for cb in range(4):
                                nc.tensor.matmul(
                                    cb_ps[cb],
                                    lhsT=wdq_t[:, b,
                                               cb * 128:(cb + 1) * 128],
                                    rhs=xt_tiles[k2][:, b, :],
                                    start=(k == 0), stop=(k == KH - 1))
                        if p == 0 and k2 == 1:
                            emit_krope_tail()
                        if p > 0 and k2 == 2:
                            emit_ssq(p - 1)
                    sq_tiles[p] = []
                    for cb in range(4):
                        kq = p * 4 + cb
                        nc.vector.tensor_copy(
                            out=cqT[:, kq, :], in_=cb_ps[cb])
                        sq = p_sq.tile([128, 512], BF16, tag="sq", name="sq")
                        nc.scalar.activation(out=sq, in_=cb_ps[cb], func=Sq)
                        sq_tiles[p].append(sq)

                # ======== mm2: q = cq.T @ wb, original head order ========
                s4 = p_stats.tile([128, MT], F32, tag="s4", name="s4")
                cs2s = []
                for n in range(NT):
                    pat = MM2_PAT[n % 3]
                    q_ps = [p_ps.tile([128, 512], F32, tag="ps", name="ps")
                            for _ in range(MT)]
                    for k2 in range(KL2):
                        wb_t = p_wb.tile([128, 2, NW], BF16, tag="wb",
                                         name="wb_t")
                        nc.sync.dma_start(
                            out=wb_t,
                            in_=wb.ap()[(n * KL2 + k2) * 128:
                                        (n * KL2 + k2 + 1) * 128, :]
                            .rearrange("q (b c) -> q b c", c=NW))
                        if n == 0 and k2 == 2:
                            emit_ssq(2)
                            # s4 = 1/sqrt(ssq/L + eps); prescale cos/sin
                            nc.scalar.activation(
                                out=s4, in_=ssq_sb, func=Sqrt,
                                bias=eps_t, scale=1.0 / L)
                            nc.vector.reciprocal(out=s4, in_=s4)
                            for m in range(MT):
                                css = p_cs.tile([128, 384], F32,
                                                tag=f"css{m}", name=f"css{m}",
                                                bufs=2)
                                nc.vector.tensor_scalar_mul(
                                    out=css, in0=cs_sb[m],
                                    scalar1=s4[:, m:m + 1])
                                cs2s.append(css)
                        for b in range(2):
                            k = 2 * k2 + b
                            for m in range(MT):
                                nc.tensor.matmul(
                                    q_ps[m],
                                    lhsT=cqT[:, k, m * 128:(m + 1) * 128],
                                    rhs=wb_t[:, b, :],
                                    start=(k == 0), stop=(k == KL - 1))
                    for m in range(MT):
                        qo = p_qout.tile([128, NW], F32, tag="q", name="qo")
                        qp = q_ps[m]
                        for base, nb, w in pat["nope"]:
                            nc.vector.tensor_scalar_mul(
                                out=_blocks(qo, base, nb, w),
                                in0=_blocks(qp, base, nb, w),
                                scalar1=s4[:, m:m + 1])
                        rb, nr = pat["rope"]
                        x1 = _blocks(qp, rb, nr, 32)
                        x2 = _blocks(qp, rb + 32, nr, 32)
                        o1 = _blocks(qo, rb, nr, 32)
                        o2 = _blocks(qo, rb + 32, nr, 32)
                        csm = cs2s[m]
                        c1 = _blocks(csm, 0, nr, 32, stride=32)
                        c2 = _blocks(csm, 96, nr, 32, stride=32)
                        s1 = _blocks(csm, 192, nr, 32, stride=32)
                        s2 = _blocks(csm, 288, nr, 32, stride=32)
                        ta = p_tmp.tile([128, 96], F32, tag="ta", name="ta")
                        tb = p_tmp.tile([128, 96], F32, tag="tb", name="tb")
                        va = _blocks(ta, 0, nr, 32, stride=32)
                        vb = _blocks(tb, 0, nr, 32, stride=32)
                        nc.vector.tensor_mul(out=va, in0=x1, in1=c1)
                        nc.vector.tensor_mul(out=vb, in0=x2, in1=s1)
                        nc.vector.tensor_sub(out=o1, in0=va, in1=vb)
                        ta2 = p_tmp.tile([128, 96], F32, tag="ta", name="ta")
                        tb2 = p_tmp.tile([128, 96], F32, tag="tb", name="tb")
                        va2 = _blocks(ta2, 0, nr, 32, stride=32)
                        vb2 = _blocks(tb2, 0, nr, 32, stride=32)
                        nc.vector.tensor_mul(out=va2, in0=x2, in1=c2)
                        nc.vector.tensor_mul(out=vb2, in0=x1, in1=s2)
                        nc.vector.tensor_add(out=o2, in0=va2, in1=vb2)
                        nc.sync.dma_start(
                            out=out_ap[m * 128:(m + 1) * 128,
                                       n * NW:(n + 1) * NW],
                            in_=qo)
    if split:
        split_multi_waits(nc)
    return nc


def prep_inputs(token_x, wq_a, wq_b, wkv, rope_cos, rope_sin, gamma_cq,
                gamma_ckv):
    """Host-side sharding + layout prep. Returns in_maps for the 8 cores."""
    bf16 = mybir.dt.np(BF16)
    # wkv -> per-k2 tiles [KH2, 128, 2, 576], flattened to 2D
    wdkv = (wkv.astype(np.float32).astype(bf16)
            .reshape(KH2, 2, 128, 576).transpose(0, 2, 1, 3)
            .reshape(KH2 * 128, 2 * 576))
    wdkv = np.ascontiguousarray(wdkv)
    # wq_a -> per (phase, k2) tiles [3, KH2, 128, 2, 512]
    wdq = (wq_a.astype(np.float32).astype(bf16)
           .reshape(KH2, 2, 128, 3, 512).transpose(3, 0, 2, 1, 4)
           .reshape(3 * KH2 * 128, 2 * 512))
    wdq = np.ascontiguousarray(wdq)
    # wq_b * gamma_cq -> per (n, k2) tiles [NT, KL2, 128, 2, 384], orig order
    wbs = wq_b.astype(np.float32) * gamma_cq.astype(np.float32)[:, None]
    wbt = (wbs.astype(bf16)
           .reshape(KL2, 2, 128, NT, NW).transpose(3, 0, 2, 1, 4)
           .reshape(NT * KL2 * 128, 2 * NW))
    wbt = np.ascontiguousarray(wbt)
    gkv = np.ascontiguousarray(gamma_ckv.astype(np.float32))
    cos = rope_cos.astype(np.float32)
    sin = rope_sin.astype(np.float32)
    c1, c2 = cos[:, 0:32], cos[:, 32:64]
    s1, s2 = sin[:, 0:32], sin[:, 32:64]
    cs_full = np.concatenate([c1, c1, c1, c2, c2, c2,
                              s1, s1, s1, s2, s2, s2], axis=1)
    in_maps = []
    for c in range(N_CORES):
        sl = slice(c * TC, (c + 1) * TC)
        xt = np.ascontiguousarray(token_x[sl].T).astype(bf16)      # [H, TC]
        cs = np.ascontiguousarray(cs_full[sl])                     # [TC, 256]
        in_maps.append({"xt": xt, "wdkv": wdkv, "wdq": wdq, "wb": wbt,
                        "cs": cs, "gkv": gkv})
    return in_maps


def kernel(token_x, wq_a, wq_b, wkv, rope_cos, rope_sin, gamma_cq, gamma_ckv):
    token_x, wq_a, wq_b, wkv, rope_cos, rope_sin, gamma_cq, gamma_ckv = (
        np.asarray(a) for a in (token_x, wq_a, wq_b, wkv, rope_cos, rope_sin,
                                gamma_cq, gamma_ckv))
    in_maps = prep_inputs(token_x, wq_a, wq_b, wkv, rope_cos, rope_sin,
                          gamma_cq, gamma_ckv)
    nc = build_nc()
    res = run_bass_kernel_spmd(nc, in_maps, list(range(N_CORES)))
    return np.concatenate([res.results[c]["out"] for c in range(N_CORES)],
                          axis=0)
